# revision 25
# baseline (speedup 1.0000x reference)
"""BitLinear (1.58-bit) kernel for Trainium2, 8-core data-parallel SPMD.

Reference op: out = sign(x) @ ternarize(W).T where
  ternarize(W) = sign(W) * min(round(|W| / gamma), 1), gamma = mean(|W|) + 1e-6.

Strategy (per sharding hint: data-parallel over batch*seq, replicate ternary W):
  - Host: ternarize W once (the "small 2048x2048 ternary weight" of the hint),
    transpose to [in, out] and pack as fp8e4 (values -1/0/+1 are exact in fp8).
    Shard x by rows (batch*seq) across the 8 cores; pre-transpose each shard to
    [in, rows] so the contraction dim lands on SBUF partitions with contiguous
    DMA lines.
  - Device (per core): DMA x^T chunks (f32), compute sign() on the Scalar
    engine straight to fp8, then a dense fp8 DoubleRow matmul (2 MACs/cell/cyc)
    accumulating in PSUM f32.  Products are +-1 and row sums <= 2048 so fp32
    accumulation is exact.
  - Host: concatenate the 8 output shards.

Layout: contraction index i in [0, 2048) is split as i = kc*256 + j*128 + p
(kc = 256-wide chunk, j = DoubleRow pair slot, p = SBUF partition).  Both
operands are stored [128, KC, 2, N] in SBUF and sliced to the 3D
[128 part, 2, N] APs that MatmulPerfMode.DoubleRow requires.
"""

import numpy as np
import ml_dtypes

import concourse.bass as bass
import concourse.bacc as bacc
import concourse.mybir as mybir
from concourse.tile import TileContext
from concourse.bass_utils import run_bass_kernel_spmd

FP8 = ml_dtypes.float8_e4m3  # maps to mybir.dt.float8e4

N_CORES = 8
EPS = 1e-6

# Full-problem shapes (hardcoded per harness contract).
B, S, I_DIM, O_DIM = 4, 4096, 2048, 2048
M_TOT = B * S                 # 16384 rows
M_PER = M_TOT // N_CORES      # 2048 rows per core


def build_program(m_per: int, k_dim: int, o_dim: int) -> bass.Bass:
    """Per-core SPMD program: out[m, o] = sign(x)[m, :] @ Wq[o, :].T.

    DRAM inputs:
      xt : [KC, 128, 2, m_per] f32   (x^T, i = kc*256 + j*128 + p)
      wt : [KC, 128, 2, o_dim] fp8e4 (Wq^T, same i layout)
    DRAM output:
      out: [m_per, o_dim] f32
    """
    KC = k_dim // 256          # 256-wide contraction chunks
    MT = m_per // 128          # output row tiles
    OT = o_dim // 512          # output col chunks (one PSUM bank each)
    assert k_dim % 256 == 0 and m_per % 128 == 0 and o_dim % 512 == 0

    # Bacc (not plain Bass): its finalize() runs generate_event_semaphores,
    # which splits multi-waits to the HW limit of 1 wait per instruction.
    nc = bacc.Bacc()
    # x travels as the high byte of its bf16 encoding (sign + 7 exponent
    # bits) — a pure byte-slice of the input that halves x traffic and is
    # exact for the sign() this op needs.
    xt = nc.declare_dram_parameter(
        "xt", [KC, 128, 2, m_per], mybir.dt.uint8, isOutput=False)
    wt = nc.declare_dram_parameter(
        "wt", [KC, 128, 2, o_dim], mybir.dt.float8e4, isOutput=False)
    # f16 output: every value is an integer in [-2048, 2048], exact in f16;
    # the host casts back to f32.  Halves the output DMA traffic.
    out = nc.declare_dram_parameter(
        "out", [m_per, o_dim], mybir.dt.float16, isOutput=True)

    with TileContext(nc) as tc:
        with (
            tc.tile_pool(name="wq", bufs=1) as wq_pool,
            tc.tile_pool(name="xs", bufs=1) as xs_pool,
            tc.tile_pool(name="xraw", bufs=1) as xraw_pool,
            tc.tile_pool(name="psum", bufs=4, space="PSUM") as psum_pool,
            tc.tile_pool(name="osb", bufs=3) as out_pool,
        ):
            # x^T high-byte chunks on the SP queue (PE's startup critical
            # path); write-once staging (bufs=1, disjoint slices) keeps every
            # HWDGE DMA at <=1 embedded sync wait (walrus limit).
            xr_sb = xraw_pool.tile([128, KC, 2, m_per], mybir.dt.uint8)
            xs_sb = xs_pool.tile([128, KC, 2, m_per], mybir.dt.uint8)
            for kc in range(KC):
                nc.sync.dma_start(out=xr_sb[:, kc], in_=xt[kc])
            # Quantized weight, fully SBUF-resident: 32 KB/partition (fp8),
            # in kc-pair DMAs on the ACT HWDGE queue, in PE consumption order.
            wq_sb = wq_pool.tile([128, KC, 2, o_dim], mybir.dt.float8e4)
            for q in range(0, KC, 2):
                qe = min(q + 2, KC)
                nc.scalar.dma_start(
                    out=wq_sb[:, q:qe],
                    in_=wt[q:qe].rearrange("k p two o -> p k two o"))

            # One-pass sign to fp8 {+1, -1}, split across DVE and ACT so the
            # chunk chain finishes in half the time:
            #   DVE (even kc): byte ops — (hi & 0x80) | 0x38 = fp8 bits of
            #     sign(x).
            #   ACT (odd kc): hi >= 0x80 iff x < 0, so Sign(127.5 - hi) =
            #     sign(x) computed numerically from the uint8 byte.
            sbias = wq_pool.tile([128, 1], mybir.dt.float32, name="sbias")
            nc.gpsimd.memset(sbias, 127.5)
            for kc in range(KC):
                if kc % 2 == 0:
                    nc.vector.tensor_scalar(
                        out=xs_sb[:, kc], in0=xr_sb[:, kc],
                        scalar1=0x80, scalar2=0x38,
                        op0=mybir.AluOpType.bitwise_and,
                        op1=mybir.AluOpType.bitwise_or)
                else:
                    nc.scalar.activation(
                        out=xs_sb[:, kc].bitcast(mybir.dt.float8e4),
                        in_=xr_sb[:, kc],
                        func=mybir.ActivationFunctionType.Sign,
                        bias=sbias, scale=-1.0)

            # PE warmup: dummy matmuls on memset scratch keep the PE busy
            # through the HAM activity window while the first x chunk lands,
            # so real matmuls start at the 2.4 GHz warm clock.
            wu_a = wq_pool.tile([128, 2, 128], mybir.dt.float8e4)
            wu_b = wq_pool.tile([128, 2, 512], mybir.dt.float8e4)
            nc.gpsimd.memset(wu_a, 0.0)
            nc.gpsimd.memset(wu_b, 0.0)
            wu_ps = psum_pool.tile([128, 512], mybir.dt.float32,
                                   name="wu_ps", tag="ps")
            for _ in range(12):
                nc.tensor.matmul(wu_ps, wu_a, wu_b, start=True, stop=True,
                                 perf_mode=mybir.MatmulPerfMode.DoubleRow)

            # Dense fp8 DoubleRow matmul: lhsT = xs (stationary), rhs = wq.
            # 2-bank PSUM half-units (bufs=4) release banks mid-mi so the
            # copy+store chain hides under the next unit's matmuls.
            n_units = max(OT // 2, 1)
            banks_per_unit = OT // n_units
            uw = banks_per_unit * 512
            for mi in range(MT):
                ot = out_pool.tile([128, o_dim], mybir.dt.float16, tag="ot")
                for half in range(n_units):
                    ps = psum_pool.tile([128, uw], mybir.dt.float32,
                                        name="ps", tag="ps")
                    for kc in range(KC):
                        lhsT = xs_sb[:, kc, :, bass.ts(mi, 128)].bitcast(
                            mybir.dt.float8e4)                    # [128,2,128]
                        for oi in range(banks_per_unit):
                            o0 = (banks_per_unit * half + oi) * 512
                            rhs = wq_sb[:, kc, :, o0:o0 + 512]    # [128,2,512]
                            nc.tensor.matmul(
                                ps[:, bass.ts(oi, 512)], lhsT, rhs,
                                start=(kc == 0), stop=(kc == KC - 1),
                                perf_mode=mybir.MatmulPerfMode.DoubleRow)
                    # psum -> sbuf, f32 -> f16 (exact); alternate DVE / ACT.
                    dst = ot[:, half * uw:(half + 1) * uw]
                    if half % 2 == 0:
                        nc.vector.tensor_copy(dst, ps)
                    else:
                        nc.scalar.copy(dst, ps)
                # one 0.5 MB store per mi on the SP HWDGE queue (idle after
                # the 8 input loads).
                nc.sync.dma_start(out=out[bass.ts(mi, 128)], in_=ot)

    # run_bass_via_pjrt does not finalize prebuilt modules; Bacc.finalize()
    # runs compile() (event-semaphore wait splitting, reg alloc, fusion).
    nc.finalize()
    return nc


def ternarize_host(weight: np.ndarray) -> np.ndarray:
    """absmean ternarization, f64 for a faithful gamma; returns {-1,0,1} f32."""
    w = weight.astype(np.float64)
    gamma = np.mean(np.abs(w)) + EPS
    return (np.sign(w) * np.minimum(np.round(np.abs(w) / gamma), 1.0)).astype(
        np.float32)


def _pack_kpj(a_t: np.ndarray) -> np.ndarray:
    """[k_dim, n] -> [KC, 128, 2, n] with i = kc*256 + j*128 + p."""
    k_dim, n = a_t.shape
    return np.ascontiguousarray(
        a_t.reshape(k_dim // 256, 2, 128, n).transpose(0, 2, 1, 3))


def prep_in_maps(x: np.ndarray, weight: np.ndarray) -> list[dict]:
    wq = ternarize_host(weight)                    # [o, i] ternary
    wt = _pack_kpj(np.ascontiguousarray(wq.T)).astype(FP8)  # [KC,128,2,o] fp8
    xf = x.reshape(M_TOT, I_DIM)
    in_maps = []
    for c in range(N_CORES):
        sh = xf[c * M_PER:(c + 1) * M_PER]         # [m_per, i]
        xb = _pack_kpj(np.ascontiguousarray(sh.T.astype(np.float32))).astype(
            ml_dtypes.bfloat16)  # bf16 is sign-exact for f32 normals
        # high byte of bf16: sign + 7 exponent bits — all sign() needs
        xt = np.ascontiguousarray((xb.view(np.uint16) >> 8).astype(np.uint8))
        in_maps.append({"xt": xt, "wt": wt})
    return in_maps


_PROGRAM_CACHE: dict = {}


def _get_program() -> bass.Bass:
    key = (M_PER, I_DIM, O_DIM)
    if key not in _PROGRAM_CACHE:
        _PROGRAM_CACHE[key] = build_program(*key)
    return _PROGRAM_CACHE[key]


def _gather(results: list[dict]) -> np.ndarray:
    full = np.concatenate([np.asarray(r["out"]) for r in results], axis=0)
    return np.ascontiguousarray(full.reshape(B, S, O_DIM).astype(np.float32))


def kernel(x: np.ndarray, weight: np.ndarray) -> np.ndarray:
    nc = _get_program()
    in_maps = prep_in_maps(np.asarray(x), np.asarray(weight))
    res = run_bass_kernel_spmd(nc, in_maps, core_ids=list(range(N_CORES)))
    return _gather(res.results)


def kernel_traced(x: np.ndarray, weight: np.ndarray, **trace_kw):
    """Like kernel() but returns (output, BassKernelResults) with a trace."""
    nc = _get_program()
    in_maps = prep_in_maps(np.asarray(x), np.asarray(weight))
    res = run_bass_kernel_spmd(
        nc, in_maps, core_ids=list(range(N_CORES)), trace=True, **trace_kw)
    return _gather(res.results), res


# revision 26
# speedup vs baseline: 1.0281x; 1.0281x over previous
"""BitLinear (1.58-bit) kernel for Trainium2, 8-core data-parallel SPMD.

Reference op: out = sign(x) @ ternarize(W).T where
  ternarize(W) = sign(W) * min(round(|W| / gamma), 1), gamma = mean(|W|) + 1e-6.

Strategy (per sharding hint: data-parallel over batch*seq, replicate ternary W):
  - Host: ternarize W once (the "small 2048x2048 ternary weight" of the hint),
    transpose to [in, out] and pack as fp8e4 (values -1/0/+1 are exact in fp8).
    Shard x by rows (batch*seq) across the 8 cores; pre-transpose each shard to
    [in, rows] so the contraction dim lands on SBUF partitions with contiguous
    DMA lines.
  - Device (per core): DMA x^T chunks (f32), compute sign() on the Scalar
    engine straight to fp8, then a dense fp8 DoubleRow matmul (2 MACs/cell/cyc)
    accumulating in PSUM f32.  Products are +-1 and row sums <= 2048 so fp32
    accumulation is exact.
  - Host: concatenate the 8 output shards.

Layout: contraction index i in [0, 2048) is split as i = kc*256 + j*128 + p
(kc = 256-wide chunk, j = DoubleRow pair slot, p = SBUF partition).  Both
operands are stored [128, KC, 2, N] in SBUF and sliced to the 3D
[128 part, 2, N] APs that MatmulPerfMode.DoubleRow requires.
"""

import numpy as np
import ml_dtypes

import concourse.bass as bass
import concourse.bacc as bacc
import concourse.mybir as mybir
from concourse.tile import TileContext
from concourse.bass_utils import run_bass_kernel_spmd

FP8 = ml_dtypes.float8_e4m3  # maps to mybir.dt.float8e4

N_CORES = 8
EPS = 1e-6

# Full-problem shapes (hardcoded per harness contract).
B, S, I_DIM, O_DIM = 4, 4096, 2048, 2048
M_TOT = B * S                 # 16384 rows
M_PER = M_TOT // N_CORES      # 2048 rows per core


def build_program(m_per: int, k_dim: int, o_dim: int) -> bass.Bass:
    """Per-core SPMD program: out[m, o] = sign(x)[m, :] @ Wq[o, :].T.

    DRAM inputs:
      xt : [KC, 128, 2, m_per] f32   (x^T, i = kc*256 + j*128 + p)
      wt : [KC, 128, 2, o_dim] fp8e4 (Wq^T, same i layout)
    DRAM output:
      out: [m_per, o_dim] f32
    """
    KC = k_dim // 256          # 256-wide contraction chunks
    MT = m_per // 128          # output row tiles
    OT = o_dim // 512          # output col chunks (one PSUM bank each)
    assert k_dim % 256 == 0 and m_per % 128 == 0 and o_dim % 512 == 0

    # Bacc (not plain Bass): its finalize() runs generate_event_semaphores,
    # which splits multi-waits to the HW limit of 1 wait per instruction.
    nc = bacc.Bacc()
    # x travels as the high byte of its bf16 encoding (sign + 7 exponent
    # bits) — a pure byte-slice of the input that halves x traffic and is
    # exact for the sign() this op needs.
    xt = nc.declare_dram_parameter(
        "xt", [KC, 128, 2, m_per], mybir.dt.uint8, isOutput=False)
    wt = nc.declare_dram_parameter(
        "wt", [KC, 128, 2, o_dim], mybir.dt.float8e4, isOutput=False)
    # f16 output: every value is an integer in [-2048, 2048], exact in f16;
    # the host casts back to f32.  Halves the output DMA traffic.
    out = nc.declare_dram_parameter(
        "out", [m_per, o_dim], mybir.dt.float16, isOutput=True)

    with TileContext(nc) as tc:
        with (
            tc.tile_pool(name="wq", bufs=1) as wq_pool,
            tc.tile_pool(name="xs", bufs=1) as xs_pool,
            tc.tile_pool(name="xraw", bufs=1) as xraw_pool,
            tc.tile_pool(name="psum", bufs=4, space="PSUM") as psum_pool,
            tc.tile_pool(name="osb", bufs=3) as out_pool,
        ):
            # x^T high-byte chunks on the SP queue (PE's startup critical
            # path); write-once staging (bufs=1, disjoint slices) keeps every
            # HWDGE DMA at <=1 embedded sync wait (walrus limit).
            xr_sb = xraw_pool.tile([128, KC, 2, m_per], mybir.dt.uint8)
            xs_sb = xs_pool.tile([128, KC, 2, m_per], mybir.dt.uint8)
            for kc in range(KC):
                nc.sync.dma_start(out=xr_sb[:, kc], in_=xt[kc])
            # Quantized weight, fully SBUF-resident: 32 KB/partition (fp8),
            # in kc-pair DMAs on the ACT HWDGE queue, in PE consumption order.
            wq_sb = wq_pool.tile([128, KC, 2, o_dim], mybir.dt.float8e4)
            for q in range(0, KC, 2):
                qe = min(q + 2, KC)
                nc.scalar.dma_start(
                    out=wq_sb[:, q:qe],
                    in_=wt[q:qe].rearrange("k p two o -> p k two o"))

            # One-pass sign to fp8 {+1, -1} on DVE, 4 bytes per lane-cycle:
            # view the hi bytes as u32 and compute (v & 0x80808080) |
            # 0x38383838 — each byte becomes the fp8e4 encoding of sign(x).
            # Two m-halves per chunk so mi=0's matmuls unblock after the
            # first half (subtile deps).
            mh = m_per // 2
            for kc in range(KC):
                for hx in range(2):
                    src = xr_sb[:, kc, :, hx * mh:(hx + 1) * mh]
                    dst = xs_sb[:, kc, :, hx * mh:(hx + 1) * mh]
                    nc.vector.tensor_scalar(
                        out=dst.bitcast(mybir.dt.uint32),
                        in0=src.bitcast(mybir.dt.uint32),
                        scalar1=0x80808080, scalar2=0x38383838,
                        op0=mybir.AluOpType.bitwise_and,
                        op1=mybir.AluOpType.bitwise_or)

            # PE warmup: dummy matmuls on memset scratch keep the PE busy
            # through the HAM activity window while the first x chunk lands,
            # so real matmuls start at the 2.4 GHz warm clock.
            wu_a = wq_pool.tile([128, 2, 128], mybir.dt.float8e4)
            wu_b = wq_pool.tile([128, 2, 512], mybir.dt.float8e4)
            nc.gpsimd.memset(wu_a, 0.0)
            nc.gpsimd.memset(wu_b, 0.0)
            wu_ps = psum_pool.tile([128, 512], mybir.dt.float32,
                                   name="wu_ps", tag="ps")
            for _ in range(12):
                nc.tensor.matmul(wu_ps, wu_a, wu_b, start=True, stop=True,
                                 perf_mode=mybir.MatmulPerfMode.DoubleRow)

            # Dense fp8 DoubleRow matmul: lhsT = xs (stationary), rhs = wq.
            # 2-bank PSUM half-units (bufs=4) release banks mid-mi so the
            # copy+store chain hides under the next unit's matmuls.
            n_units = max(OT // 2, 1)
            banks_per_unit = OT // n_units
            uw = banks_per_unit * 512
            for mi in range(MT):
                ot = out_pool.tile([128, o_dim], mybir.dt.float16, tag="ot")
                for half in range(n_units):
                    ps = psum_pool.tile([128, uw], mybir.dt.float32,
                                        name="ps", tag="ps")
                    for kc in range(KC):
                        lhsT = xs_sb[:, kc, :, bass.ts(mi, 128)].bitcast(
                            mybir.dt.float8e4)                    # [128,2,128]
                        for oi in range(banks_per_unit):
                            o0 = (banks_per_unit * half + oi) * 512
                            rhs = wq_sb[:, kc, :, o0:o0 + 512]    # [128,2,512]
                            nc.tensor.matmul(
                                ps[:, bass.ts(oi, 512)], lhsT, rhs,
                                start=(kc == 0), stop=(kc == KC - 1),
                                perf_mode=mybir.MatmulPerfMode.DoubleRow)
                    # psum -> sbuf, f32 -> f16 (exact); alternate DVE / ACT.
                    dst = ot[:, half * uw:(half + 1) * uw]
                    if half % 2 == 0:
                        nc.vector.tensor_copy(dst, ps)
                    else:
                        nc.scalar.copy(dst, ps)
                # one 0.5 MB store per mi on the SP HWDGE queue (idle after
                # the 8 input loads).
                nc.sync.dma_start(out=out[bass.ts(mi, 128)], in_=ot)

    # run_bass_via_pjrt does not finalize prebuilt modules; Bacc.finalize()
    # runs compile() (event-semaphore wait splitting, reg alloc, fusion).
    nc.finalize()
    return nc


def ternarize_host(weight: np.ndarray) -> np.ndarray:
    """absmean ternarization, f64 for a faithful gamma; returns {-1,0,1} f32."""
    w = weight.astype(np.float64)
    gamma = np.mean(np.abs(w)) + EPS
    return (np.sign(w) * np.minimum(np.round(np.abs(w) / gamma), 1.0)).astype(
        np.float32)


def _pack_kpj(a_t: np.ndarray) -> np.ndarray:
    """[k_dim, n] -> [KC, 128, 2, n] with i = kc*256 + j*128 + p."""
    k_dim, n = a_t.shape
    return np.ascontiguousarray(
        a_t.reshape(k_dim // 256, 2, 128, n).transpose(0, 2, 1, 3))


def prep_in_maps(x: np.ndarray, weight: np.ndarray) -> list[dict]:
    wq = ternarize_host(weight)                    # [o, i] ternary
    wt = _pack_kpj(np.ascontiguousarray(wq.T)).astype(FP8)  # [KC,128,2,o] fp8
    xf = x.reshape(M_TOT, I_DIM)
    in_maps = []
    for c in range(N_CORES):
        sh = xf[c * M_PER:(c + 1) * M_PER]         # [m_per, i]
        xb = _pack_kpj(np.ascontiguousarray(sh.T.astype(np.float32))).astype(
            ml_dtypes.bfloat16)  # bf16 is sign-exact for f32 normals
        # high byte of bf16: sign + 7 exponent bits — all sign() needs
        xt = np.ascontiguousarray((xb.view(np.uint16) >> 8).astype(np.uint8))
        in_maps.append({"xt": xt, "wt": wt})
    return in_maps


_PROGRAM_CACHE: dict = {}


def _get_program() -> bass.Bass:
    key = (M_PER, I_DIM, O_DIM)
    if key not in _PROGRAM_CACHE:
        _PROGRAM_CACHE[key] = build_program(*key)
    return _PROGRAM_CACHE[key]


def _gather(results: list[dict]) -> np.ndarray:
    full = np.concatenate([np.asarray(r["out"]) for r in results], axis=0)
    return np.ascontiguousarray(full.reshape(B, S, O_DIM).astype(np.float32))


def kernel(x: np.ndarray, weight: np.ndarray) -> np.ndarray:
    nc = _get_program()
    in_maps = prep_in_maps(np.asarray(x), np.asarray(weight))
    res = run_bass_kernel_spmd(nc, in_maps, core_ids=list(range(N_CORES)))
    return _gather(res.results)


def kernel_traced(x: np.ndarray, weight: np.ndarray, **trace_kw):
    """Like kernel() but returns (output, BassKernelResults) with a trace."""
    nc = _get_program()
    in_maps = prep_in_maps(np.asarray(x), np.asarray(weight))
    res = run_bass_kernel_spmd(
        nc, in_maps, core_ids=list(range(N_CORES)), trace=True, **trace_kw)
    return _gather(res.results), res


# revision 30
# speedup vs baseline: 1.0460x; 1.0174x over previous
"""BitLinear (1.58-bit) kernel for Trainium2, 8-core data-parallel SPMD.

Reference op: out = sign(x) @ ternarize(W).T where
  ternarize(W) = sign(W) * min(round(|W| / gamma), 1), gamma = mean(|W|) + 1e-6.

Strategy (per sharding hint: data-parallel over batch*seq, replicate ternary W):
  - Host: ternarize W once (the "small 2048x2048 ternary weight" of the hint),
    transpose to [in, out] and pack as fp8e4 (values -1/0/+1 are exact in fp8).
    Shard x by rows (batch*seq) across the 8 cores; pre-transpose each shard to
    [in, rows] so the contraction dim lands on SBUF partitions with contiguous
    DMA lines.
  - Device (per core): DMA x^T chunks (f32), compute sign() on the Scalar
    engine straight to fp8, then a dense fp8 DoubleRow matmul (2 MACs/cell/cyc)
    accumulating in PSUM f32.  Products are +-1 and row sums <= 2048 so fp32
    accumulation is exact.
  - Host: concatenate the 8 output shards.

Layout: contraction index i in [0, 2048) is split as i = kc*256 + j*128 + p
(kc = 256-wide chunk, j = DoubleRow pair slot, p = SBUF partition).  Both
operands are stored [128, KC, 2, N] in SBUF and sliced to the 3D
[128 part, 2, N] APs that MatmulPerfMode.DoubleRow requires.
"""

import numpy as np
import ml_dtypes

import concourse.bass as bass
import concourse.bacc as bacc
import concourse.mybir as mybir
from concourse.tile import TileContext
from concourse.bass_utils import run_bass_kernel_spmd

FP8 = ml_dtypes.float8_e4m3  # maps to mybir.dt.float8e4

N_CORES = 8
EPS = 1e-6

# Full-problem shapes (hardcoded per harness contract).
B, S, I_DIM, O_DIM = 4, 4096, 2048, 2048
M_TOT = B * S                 # 16384 rows
M_PER = M_TOT // N_CORES      # 2048 rows per core


def build_program(m_per: int, k_dim: int, o_dim: int) -> bass.Bass:
    """Per-core SPMD program: out[m, o] = sign(x)[m, :] @ Wq[o, :].T.

    DRAM inputs:
      xt : [KC, 128, 2, m_per] f32   (x^T, i = kc*256 + j*128 + p)
      wt : [KC, 128, 2, o_dim] fp8e4 (Wq^T, same i layout)
    DRAM output:
      out: [m_per, o_dim] f32
    """
    KC = k_dim // 256          # 256-wide contraction chunks
    MT = m_per // 128          # output row tiles
    OT = o_dim // 512          # output col chunks (one PSUM bank each)
    assert k_dim % 256 == 0 and m_per % 128 == 0 and o_dim % 512 == 0

    # Bacc (not plain Bass): its finalize() runs generate_event_semaphores,
    # which splits multi-waits to the HW limit of 1 wait per instruction.
    nc = bacc.Bacc()
    # x travels as the high byte of its bf16 encoding (sign + 7 exponent
    # bits) — a pure byte-slice of the input that halves x traffic and is
    # exact for the sign() this op needs.
    xt = nc.declare_dram_parameter(
        "xt", [KC, 128, 2, m_per], mybir.dt.uint8, isOutput=False)
    wt = nc.declare_dram_parameter(
        "wt", [KC, 128, 2, o_dim], mybir.dt.float8e4, isOutput=False)
    # f16 output: every value is an integer in [-2048, 2048], exact in f16;
    # the host casts back to f32.  Halves the output DMA traffic.
    out = nc.declare_dram_parameter(
        "out", [m_per, o_dim], mybir.dt.float16, isOutput=True)

    with TileContext(nc) as tc:
        with (
            tc.tile_pool(name="wq", bufs=1) as wq_pool,
            tc.tile_pool(name="xs", bufs=1) as xs_pool,
            tc.tile_pool(name="xraw", bufs=1) as xraw_pool,
            tc.tile_pool(name="psum", bufs=4, space="PSUM") as psum_pool,
            tc.tile_pool(name="osb", bufs=3) as out_pool,
        ):
            # x^T high-byte chunks on the SP queue (PE's startup critical
            # path); write-once staging (bufs=1, disjoint slices) keeps every
            # HWDGE DMA at <=1 embedded sync wait (walrus limit).
            xr_sb = xraw_pool.tile([128, KC, 2, m_per], mybir.dt.uint8)
            xs_sb = xs_pool.tile([128, KC, 2, m_per], mybir.dt.uint8)
            mh = m_per // 2
            for kc in range(KC):
                if kc < 2:
                    # first chunks in m-halves so sign/matmul start earlier
                    for hx in range(2):
                        nc.sync.dma_start(
                            out=xr_sb[:, kc, :, hx * mh:(hx + 1) * mh],
                            in_=xt[kc, :, :, hx * mh:(hx + 1) * mh])
                else:
                    nc.sync.dma_start(out=xr_sb[:, kc], in_=xt[kc])
            # Quantized weight, fully SBUF-resident: 32 KB/partition (fp8),
            # on the ACT HWDGE queue in PE consumption order, first two
            # chunks solo for an earlier matmul start.
            wq_sb = wq_pool.tile([128, KC, 2, o_dim], mybir.dt.float8e4)
            wq_groups = [(0, 1), (1, 2)] + [
                (q, min(q + 2, KC)) for q in range(2, KC, 2)]
            for q0, q1 in wq_groups:
                if q0 >= KC:
                    continue
                nc.scalar.dma_start(
                    out=wq_sb[:, q0:q1],
                    in_=wt[q0:q1].rearrange("k p two o -> p k two o"))

            # One-pass sign to fp8 {+1, -1} on DVE, 4 bytes per lane-cycle:
            # view the hi bytes as u32 and compute (v & 0x80808080) |
            # 0x38383838 — each byte becomes the fp8e4 encoding of sign(x).
            # Two m-halves per chunk so mi=0's matmuls unblock after the
            # first half (subtile deps).
            for kc in range(KC):
                for hx in range(2):
                    src = xr_sb[:, kc, :, hx * mh:(hx + 1) * mh]
                    dst = xs_sb[:, kc, :, hx * mh:(hx + 1) * mh]
                    nc.vector.tensor_scalar(
                        out=dst.bitcast(mybir.dt.uint32),
                        in0=src.bitcast(mybir.dt.uint32),
                        scalar1=0x80808080, scalar2=0x38383838,
                        op0=mybir.AluOpType.bitwise_and,
                        op1=mybir.AluOpType.bitwise_or)

            # PE warmup: dummy matmuls on memset scratch keep the PE busy
            # through the HAM activity window while the first x chunk lands,
            # so real matmuls start at the 2.4 GHz warm clock.
            wu_a = wq_pool.tile([128, 2, 128], mybir.dt.float8e4)
            wu_b = wq_pool.tile([128, 2, 512], mybir.dt.float8e4)
            nc.gpsimd.memset(wu_a, 0.0)
            nc.gpsimd.memset(wu_b, 0.0)
            wu_ps = psum_pool.tile([128, 512], mybir.dt.float32,
                                   name="wu_ps", tag="ps")
            for _ in range(16):
                nc.tensor.matmul(wu_ps, wu_a, wu_b, start=True, stop=True,
                                 perf_mode=mybir.MatmulPerfMode.DoubleRow)

            # Dense fp8 DoubleRow matmul: lhsT = xs (stationary), rhs = wq.
            # 2-bank PSUM half-units (bufs=4) release banks mid-mi so the
            # copy+store chain hides under the next unit's matmuls.
            for mi in range(MT):
                # last mi: 1-bank units so the final copy chain is short
                if mi == MT - 1 and OT >= 2:
                    n_units, banks_per_unit = OT, 1
                else:
                    n_units = max(OT // 2, 1)
                    banks_per_unit = OT // n_units
                uw = banks_per_unit * 512
                ot = out_pool.tile([128, o_dim], mybir.dt.float16, tag="ot")
                for half in range(n_units):
                    ps = psum_pool.tile([128, uw], mybir.dt.float32,
                                        name="ps", tag="ps")
                    for kc in range(KC):
                        lhsT = xs_sb[:, kc, :, bass.ts(mi, 128)].bitcast(
                            mybir.dt.float8e4)                    # [128,2,128]
                        for oi in range(banks_per_unit):
                            o0 = (banks_per_unit * half + oi) * 512
                            rhs = wq_sb[:, kc, :, o0:o0 + 512]    # [128,2,512]
                            nc.tensor.matmul(
                                ps[:, bass.ts(oi, 512)], lhsT, rhs,
                                start=(kc == 0), stop=(kc == KC - 1),
                                perf_mode=mybir.MatmulPerfMode.DoubleRow)
                    # psum -> sbuf, f32 -> f16 (exact); alternate DVE / ACT.
                    dst = ot[:, half * uw:(half + 1) * uw]
                    if half % 2 == 0:
                        nc.vector.tensor_copy(dst, ps)
                    else:
                        nc.scalar.copy(dst, ps)
                # one 0.5 MB store per mi on the SP HWDGE queue (idle after
                # the 8 input loads).
                nc.sync.dma_start(out=out[bass.ts(mi, 128)], in_=ot)

    # run_bass_via_pjrt does not finalize prebuilt modules; Bacc.finalize()
    # runs compile() (event-semaphore wait splitting, reg alloc, fusion).
    nc.finalize()
    return nc


def ternarize_host(weight: np.ndarray) -> np.ndarray:
    """absmean ternarization, f64 for a faithful gamma; returns {-1,0,1} f32."""
    w = weight.astype(np.float64)
    gamma = np.mean(np.abs(w)) + EPS
    return (np.sign(w) * np.minimum(np.round(np.abs(w) / gamma), 1.0)).astype(
        np.float32)


def _pack_kpj(a_t: np.ndarray) -> np.ndarray:
    """[k_dim, n] -> [KC, 128, 2, n] with i = kc*256 + j*128 + p."""
    k_dim, n = a_t.shape
    return np.ascontiguousarray(
        a_t.reshape(k_dim // 256, 2, 128, n).transpose(0, 2, 1, 3))


def prep_in_maps(x: np.ndarray, weight: np.ndarray) -> list[dict]:
    wq = ternarize_host(weight)                    # [o, i] ternary
    wt = _pack_kpj(np.ascontiguousarray(wq.T)).astype(FP8)  # [KC,128,2,o] fp8
    xf = x.reshape(M_TOT, I_DIM)
    in_maps = []
    for c in range(N_CORES):
        sh = xf[c * M_PER:(c + 1) * M_PER]         # [m_per, i]
        xb = _pack_kpj(np.ascontiguousarray(sh.T.astype(np.float32))).astype(
            ml_dtypes.bfloat16)  # bf16 is sign-exact for f32 normals
        # high byte of bf16: sign + 7 exponent bits — all sign() needs
        xt = np.ascontiguousarray((xb.view(np.uint16) >> 8).astype(np.uint8))
        in_maps.append({"xt": xt, "wt": wt})
    return in_maps


_PROGRAM_CACHE: dict = {}


def _get_program() -> bass.Bass:
    key = (M_PER, I_DIM, O_DIM)
    if key not in _PROGRAM_CACHE:
        _PROGRAM_CACHE[key] = build_program(*key)
    return _PROGRAM_CACHE[key]


def _gather(results: list[dict]) -> np.ndarray:
    full = np.concatenate([np.asarray(r["out"]) for r in results], axis=0)
    return np.ascontiguousarray(full.reshape(B, S, O_DIM).astype(np.float32))


def kernel(x: np.ndarray, weight: np.ndarray) -> np.ndarray:
    nc = _get_program()
    in_maps = prep_in_maps(np.asarray(x), np.asarray(weight))
    res = run_bass_kernel_spmd(nc, in_maps, core_ids=list(range(N_CORES)))
    return _gather(res.results)


def kernel_traced(x: np.ndarray, weight: np.ndarray, **trace_kw):
    """Like kernel() but returns (output, BassKernelResults) with a trace."""
    nc = _get_program()
    in_maps = prep_in_maps(np.asarray(x), np.asarray(weight))
    res = run_bass_kernel_spmd(
        nc, in_maps, core_ids=list(range(N_CORES)), trace=True, **trace_kw)
    return _gather(res.results), res
